# revision 7
# baseline (speedup 1.0000x reference)
# Trainium2 Bass kernel for nn_Attention_88313117540497.
#
# Reference computation (per batch b of 128):
#   v = x_b @ Wv; conv2d of each channel's 14x14 image with 27x27 qk at
#   padding 13; y = conv_out @ Wo + bo.
#
# Algebra:
#  1. The padded 27x27 conv on 14x14 covers every pixel pair, so it is a
#     dense 196x196 map M shared across batches/channels:
#         y_b = M @ x_b @ (Wv@Wo) + bo,   W = Wv@Wo (384x384).
#  2. PE mapping (out = lhsT.T @ rhs; lhsT stationary, rhs streams N
#     cycles):
#       stage A:  G^T_b = lhsT(X_b).T @ MT    X_b in natural token-major
#                 layout, MT = M^T. 3 d-chunks x 2 v-chunks, N=196.
#       stage B:  Y^T_b = lhsT(W).T @ G^T_b   3 e-chunks x 3 d-chunks,
#                 N=196, full 128x128 array occupancy (FLOP-optimal).
#     All operands bf16: halves DMA, enables Fast Weight Load so the
#     LDWEIGHTS stream hides under the matmul stream.
#  3. Software pipeline: stage A of batch b+1 is issued before stage B of
#     batch b so the PE never stalls on the G eviction, and the PE stream
#     is gapless -> HAM clock-gate reaches 8/8 and stays there. A burst
#     of N=512 warm-up matmuls on const APs ramps HAM during the first
#     x DMA.
#  4. PSUM tiles use a 256-element column stride (chunks at 0/256/512)
#     so all three 196-wide chunks of G (or Y) sit in 2 banks without a
#     matmul output crossing a bank, and the eviction is a single
#     strided-AP op: scalar ACT copy for G (fp32->bf16), vector
#     tensor_tensor add for Y (bias fused, fp32->fp16). One op per
#     engine per batch.
#
# Sharding: data-parallel over batch, 16 batches/core, no collectives.

import numpy as np
import ml_dtypes

import concourse.bass as bass
from concourse import bacc
import concourse.mybir as mybir
import concourse.tile as tile
from concourse.bass_utils import run_bass_kernel_spmd

N_CORES = 8
B = 128
BPC = B // N_CORES      # 16 batches per core
DIM = 384
NPOS = 196
IMG = 14
KS = 27

F32 = mybir.dt.float32
BF16 = mybir.dt.bfloat16
FP16 = mybir.dt.float16
BF16_NP = ml_dtypes.bfloat16

DCH = 3                             # 128-chunks of DIM
VCHUNKS = [(0, 128), (128, 68)]     # token chunks (stage-A contraction)
YW = 3 * NPOS                       # 588 fp16 per batch in the output
PSTRIDE = 256                       # psum column stride between chunks
# progressive x-load groups: a small first group lands quickly; x is
# fully SBUF-resident (24KB/partition) so no triggers are WAR-gated
XGROUPS = [(0, 2), (2, 6), (8, 8)]
XB = 2 * DIM                        # bf16 cols per batch in the x tile
# y stores: taper the tail so the final transfer + receipt is small
YSTORES = [(0, 4), (4, 4), (8, 4), (12, 2), (14, 1), (15, 1)]
NWARM = 5


def build_program():
    nc = bacc.Bacc("TRN2", debug=False)

    # x, partition-major: [token, batch, feature] bf16 per core
    x_d = nc.dram_tensor("x", [NPOS, BPC, DIM], BF16, kind="ExternalInput")
    w_d = nc.dram_tensor("w", [DIM, DIM], BF16, kind="ExternalInput")
    # mtb: packed [128, 392 mt-chunks | 588 bias] bf16
    mtb_d = nc.dram_tensor("mtb", [128, 2 * NPOS + YW], BF16,
                           kind="ExternalInput")
    # y, e-major fp16: [partition, batch * (e-chunk, u)]
    y_d = nc.dram_tensor("y", [128, BPC * YW], FP16, kind="ExternalOutput")

    xgrp = {}
    for s0, sz in XGROUPS:
        for bb in range(s0, s0 + sz):
            xgrp[bb] = (s0, sz)

    with tile.TileContext(nc) as tc:
        with (
            tc.tile_pool(name="const", bufs=1) as const,
            tc.tile_pool(name="work", bufs=2) as work,
            tc.tile_pool(name="psum", bufs=2, space="PSUM") as psum,
        ):
            # ---- constants (scalar HWDGE; scalar is idle at start) ----
            mtb_sb = const.tile([128, 2 * NPOS + YW], BF16)
            nc.scalar.dma_start(mtb_sb[:, :], mtb_d[:, :])
            w_sb = const.tile([128, DCH * DIM], BF16)
            nc.scalar.dma_start(
                w_sb[:, :].rearrange("p (c e) -> p c e", c=DCH),
                w_d.rearrange("(c p) e -> p c e", p=128),
            )
            mt = mtb_sb[:, 0:2 * NPOS]
            bias = mtb_sb[:, 2 * NPOS:2 * NPOS + YW].rearrange(
                "p (c u) -> p c u", c=DCH)

            # ---- PE warm-up: dense N=512 bf16 matmuls ramp the HAM
            # clock gate to 8/8 while the first x group is in flight ----
            warm_c = nc.const_aps.tensor(1.0, (128, 512), BF16)
            for wi in range(NWARM):
                warm = psum.tile([128, DCH * PSTRIDE], F32, tag="g",
                                 name=f"warm{wi}")
                nc.tensor.matmul(
                    warm[0:1, 0:512], lhsT=warm_c[:, 0:1], rhs=warm_c,
                    start=True, stop=True,
                )

            # ---- x: fully resident, progressive unconditional loads ----
            x_t = const.tile([128, BPC * XB], BF16, name="xall")
            for gstart, gsize in XGROUPS:
                xv = x_t[:, gstart * XB:(gstart + gsize) * XB].rearrange(
                    "p (b c d) -> p b c d", b=gsize, c=2)
                nc.sync.dma_start(
                    xv[:, :, 0, :],
                    x_d[0:128, gstart:gstart + gsize, :])
                nc.sync.dma_start(
                    xv[0:68, :, 1, :],
                    x_d[128:NPOS, gstart:gstart + gsize, :])

            # ---- software-pipelined main loop: A(b) then B(b-1) ----
            y_t = None
            gts = {}
            for b in range(BPC + 1):
                if b < BPC:
                    xo = b * XB

                    # stage A: G^T_b (d on partitions), tokens = K
                    gp = psum.tile([128, DCH * PSTRIDE], F32, tag="g",
                                   name=f"g{b}")
                    for m in range(DCH):
                        for v, (v0, vsz) in enumerate(VCHUNKS):
                            nc.tensor.matmul(
                                gp[:, m * PSTRIDE:m * PSTRIDE + NPOS],
                                lhsT=x_t[0:vsz,
                                         xo + v * DIM + m * 128:
                                         xo + v * DIM + m * 128 + 128],
                                rhs=mt[0:vsz, v * NPOS:(v + 1) * NPOS],
                                start=(v == 0),
                                stop=(v == 1),
                            )
                    # single strided eviction fp32->bf16 on scalar
                    gt = work.tile([128, DCH * NPOS], BF16, tag="gt",
                                   bufs=3, name=f"gt{b}")
                    nc.scalar.copy(
                        gt[:, :].rearrange("p (c u) -> p c u", c=DCH),
                        gp[:, :].rearrange("p (c s) -> p c s",
                                           c=DCH)[:, :, 0:NPOS],
                    )
                    gts[b] = gt

                if b >= 1:
                    bb = b - 1        # stage B batch
                    bi = bb % 4
                    if bi == 0:
                        y_t = work.tile([128, 4 * YW], FP16, tag="y",
                                        bufs=2, name=f"y{bb // 4}")
                    gt = gts.pop(bb)

                    # stage B: Y^T_b (e on partitions), d = K, W shared
                    yp = psum.tile([128, DCH * PSTRIDE], F32, tag="yp",
                                   name=f"yp{bb}")
                    for e in range(DCH):
                        for d in range(DCH):
                            nc.tensor.matmul(
                                yp[:, e * PSTRIDE:e * PSTRIDE + NPOS],
                                lhsT=w_sb[:, d * DIM + e * 128:
                                          d * DIM + e * 128 + 128],
                                rhs=gt[:, d * NPOS:(d + 1) * NPOS],
                                start=(d == 0),
                                stop=(d == DCH - 1),
                            )
                    # single strided eviction + bias, fp32->fp16, vector
                    nc.vector.tensor_add(
                        y_t[:, bi * YW:(bi + 1) * YW].rearrange(
                            "p (c u) -> p c u", c=DCH),
                        yp[:, :].rearrange("p (c s) -> p c s",
                                           c=DCH)[:, :, 0:NPOS],
                        bias,
                    )
                    for s0, ssz in YSTORES:
                        if bb == s0 + ssz - 1:
                            nc.sync.dma_start(
                                y_d[:, s0 * YW:(s0 + ssz) * YW],
                                y_t[:, (s0 % 4) * YW:
                                    (s0 % 4 + ssz) * YW])

    nc.compile()
    return nc


_PROGRAM = None


def _get_program():
    global _PROGRAM
    if _PROGRAM is None:
        _PROGRAM = build_program()
    return _PROGRAM


def _host_prep(x, Wv, qk, Wo, bo):
    x = np.asarray(x, dtype=np.float32)
    # per-core partition-major: [core, token, batch, feature] bf16
    XC = np.ascontiguousarray(
        x.reshape(N_CORES, BPC, NPOS, DIM).transpose(0, 2, 1, 3)
    ).astype(BF16_NP)
    W = (np.asarray(Wv, np.float32) @ np.asarray(Wo, np.float32)).astype(BF16_NP)
    # MT[(u,v),(p,q)] = qk[13+u-p, 13+v-q]: conv as a 196x196 matmul
    qk2 = np.asarray(qk, np.float32).reshape(KS, KS)
    idx = (KS // 2) + np.arange(IMG)[:, None] - np.arange(IMG)[None, :]
    MT = np.ascontiguousarray(
        qk2[idx[:, None, :, None], idx[None, :, None, :]].reshape(NPOS, NPOS)
    ).astype(BF16_NP)
    bo = np.asarray(bo, np.float32)
    mtb = np.zeros((128, 2 * NPOS + YW), dtype=BF16_NP)
    mtb[:, 0:NPOS] = MT[0:128, :]
    mtb[0:68, NPOS:2 * NPOS] = MT[128:NPOS, :]
    be = bo.reshape(DCH, 128).astype(BF16_NP)    # bias[c][p] = bo[128c+p]
    for c in range(DCH):
        mtb[:, 2 * NPOS + c * NPOS:2 * NPOS + (c + 1) * NPOS] = be[c][:, None]
    return XC, W, mtb


def _unpack_core(y2):
    # y2: [128, BPC*588] fp16 -> (BPC, NPOS, DIM) f32
    a = np.asarray(y2, np.float32).reshape(128, BPC, DCH, NPOS)
    # out[b, u, e=128c+p] = a[p, b, c, u]
    return np.ascontiguousarray(
        a.transpose(1, 3, 2, 0).reshape(BPC, NPOS, DIM))


def _run(x, Wv, qk, Wo, bo, **spmd_kwargs):
    XC, W, mtb = _host_prep(x, Wv, qk, Wo, bo)
    nc = _get_program()
    in_maps = [
        {"x": XC[c], "w": W, "mtb": mtb}
        for c in range(N_CORES)
    ]
    res = run_bass_kernel_spmd(nc, in_maps, list(range(N_CORES)), **spmd_kwargs)
    y = np.concatenate(
        [_unpack_core(res.results[c]["y"]) for c in range(N_CORES)], axis=0)
    return y, res


def kernel(x, Wv, qk, Wo, bo):
    y, _ = _run(x, Wv, qk, Wo, bo)
    return y


# revision 9
# speedup vs baseline: 1.2016x; 1.2016x over previous
# Trainium2 Bass kernel for nn_Attention_88313117540497.
#
# Reference computation (per batch b of 128):
#   v = x_b @ Wv; conv2d of each channel's 14x14 image with 27x27 qk at
#   padding 13; y = conv_out @ Wo + bo.
#
# Algebra:
#  1. The padded 27x27 conv on 14x14 covers every pixel pair, so it is a
#     dense 196x196 map M shared across batches/channels:
#         y_b = M @ x_b @ (Wv@Wo) + bo,   W = Wv@Wo (384x384).
#  2. PE mapping (out = lhsT.T @ rhs; lhsT stationary, rhs streams N
#     cycles):
#       stage A:  G^T_b = lhsT(X_b).T @ MT    X_b in natural token-major
#                 layout, MT = M^T. 3 d-chunks x 2 v-chunks, N=196.
#       stage B:  Y^T_b = lhsT(W).T @ G^T_b   3 e-chunks x 3 d-chunks,
#                 N=196, full 128x128 array occupancy (FLOP-optimal).
#     All operands bf16: halves DMA, enables Fast Weight Load so the
#     LDWEIGHTS stream hides under the matmul stream.
#  3. Software pipeline: stage A of batch b+1 is issued before stage B of
#     batch b so the PE never stalls on the G eviction, and the PE stream
#     is gapless -> HAM clock-gate reaches 8/8 and stays there. A burst
#     of N=512 warm-up matmuls on const APs ramps HAM during the first
#     x DMA.
#  4. PSUM tiles use a 256-element column stride (chunks at 0/256/512)
#     so all three 196-wide chunks of G (or Y) sit in 2 banks without a
#     matmul output crossing a bank, and the eviction is a single
#     strided-AP op: scalar ACT copy for G (fp32->bf16), vector
#     tensor_tensor add for Y (bias fused, fp32->fp16). One op per
#     engine per batch.
#
# Sharding: data-parallel over batch, 16 batches/core, no collectives.

import numpy as np
import ml_dtypes

import concourse.bass as bass
from concourse import bacc
import concourse.mybir as mybir
import concourse.tile as tile
from concourse.bass_utils import run_bass_kernel_spmd

N_CORES = 8
B = 128
BPC = B // N_CORES      # 16 batches per core
DIM = 384
NPOS = 196
IMG = 14
KS = 27

F32 = mybir.dt.float32
BF16 = mybir.dt.bfloat16
FP16 = mybir.dt.float16
BF16_NP = ml_dtypes.bfloat16

DCH = 3                             # 128-chunks of DIM
VCHUNKS = [(0, 128), (128, 68)]     # token chunks (stage-A contraction)
YW = 3 * NPOS                       # 588 fp16 per batch in the output
PSTRIDE = 256                       # psum column stride between chunks
# progressive x-load groups: a small first group lands quickly; x is
# fully SBUF-resident (24KB/partition) so no triggers are WAR-gated
XGROUPS = [(0, 2), (2, 6), (8, 8)]
XB = 2 * DIM                        # bf16 cols per batch in the x tile
# y stores: taper the tail so the final transfer + receipt is small
YSTORES = [(0, 4), (4, 4), (8, 4), (12, 2), (14, 1), (15, 1)]
NWARM = 5


def build_program():
    nc = bacc.Bacc("TRN2", debug=False)

    # x, partition-major: [token, batch, feature] bf16 per core
    x_d = nc.dram_tensor("x", [NPOS, BPC, DIM], BF16, kind="ExternalInput")
    w_d = nc.dram_tensor("w", [DIM, DIM], BF16, kind="ExternalInput")
    # mtb: packed [128, 392 mt-chunks | 588 bias] bf16
    mtb_d = nc.dram_tensor("mtb", [128, 2 * NPOS + YW], BF16,
                           kind="ExternalInput")
    # y, e-major fp16: [partition, batch * (e-chunk, u)]
    y_d = nc.dram_tensor("y", [128, BPC * YW], FP16, kind="ExternalOutput")

    xgrp = {}
    for s0, sz in XGROUPS:
        for bb in range(s0, s0 + sz):
            xgrp[bb] = (s0, sz)

    with tile.TileContext(nc) as tc:
        with (
            tc.tile_pool(name="const", bufs=1) as const,
            tc.tile_pool(name="work", bufs=2) as work,
            tc.tile_pool(name="psum", bufs=2, space="PSUM") as psum,
        ):
            # ---- constants (scalar HWDGE; scalar is idle at start) ----
            mtb_sb = const.tile([128, 2 * NPOS + YW], BF16)
            nc.scalar.dma_start(mtb_sb[:, :], mtb_d[:, :])
            w_sb = const.tile([128, DCH * DIM], BF16)
            nc.scalar.dma_start(
                w_sb[:, :].rearrange("p (c e) -> p c e", c=DCH),
                w_d.rearrange("(c p) e -> p c e", p=128),
            )
            mt = mtb_sb[:, 0:2 * NPOS]
            bias = mtb_sb[:, 2 * NPOS:2 * NPOS + YW].rearrange(
                "p (c u) -> p c u", c=DCH)

            # ---- PE warm-up: dense N=512 bf16 matmuls ramp the HAM
            # clock gate to 8/8 while the first x group is in flight ----
            warm_c = nc.const_aps.tensor(1.0, (128, 512), BF16)
            for wi in range(NWARM):
                warm = psum.tile([128, DCH * PSTRIDE], F32, tag="g",
                                 name=f"warm{wi}")
                nc.tensor.matmul(
                    warm[0:1, 0:512], lhsT=warm_c[:, 0:1], rhs=warm_c,
                    start=True, stop=True,
                )

            # ---- x: fully resident, progressive unconditional loads.
            # A-chunk (tokens 0:128) and B-chunk (tokens 128:196) regions
            # are separate so each DMA is contiguous per partition ----
            x_t = const.tile([128, 2 * BPC * DIM], BF16, name="xall")
            XBOFF = BPC * DIM
            for gstart, gsize in XGROUPS:
                nc.sync.dma_start(
                    x_t[:, gstart * DIM:(gstart + gsize) * DIM].rearrange(
                        "p (b d) -> p b d", b=gsize),
                    x_d[0:128, gstart:gstart + gsize, :])
                nc.sync.dma_start(
                    x_t[0:68, XBOFF + gstart * DIM:
                        XBOFF + (gstart + gsize) * DIM].rearrange(
                        "p (b d) -> p b d", b=gsize),
                    x_d[128:NPOS, gstart:gstart + gsize, :])

            # ---- software-pipelined main loop: A(b) then B(b-1) ----
            y_t = None
            gts = {}
            for b in range(BPC + 1):
                if b < BPC:
                    # stage A: G^T_b (d on partitions), tokens = K
                    gp = psum.tile([128, DCH * PSTRIDE], F32, tag="g",
                                   name=f"g{b}")
                    for m in range(DCH):
                        for v, (v0, vsz) in enumerate(VCHUNKS):
                            xc = v * XBOFF + b * DIM + m * 128
                            nc.tensor.matmul(
                                gp[:, m * PSTRIDE:m * PSTRIDE + NPOS],
                                lhsT=x_t[0:vsz, xc:xc + 128],
                                rhs=mt[0:vsz, v * NPOS:(v + 1) * NPOS],
                                start=(v == 0),
                                stop=(v == 1),
                            )
                    # single strided eviction fp32->bf16 on scalar
                    gt = work.tile([128, DCH * NPOS], BF16, tag="gt",
                                   bufs=3, name=f"gt{b}")
                    nc.scalar.copy(
                        gt[:, :].rearrange("p (c u) -> p c u", c=DCH),
                        gp[:, :].rearrange("p (c s) -> p c s",
                                           c=DCH)[:, :, 0:NPOS],
                    )
                    gts[b] = gt

                if b >= 1:
                    bb = b - 1        # stage B batch
                    bi = bb % 4
                    if bi == 0:
                        y_t = work.tile([128, 4 * YW], FP16, tag="y",
                                        bufs=2, name=f"y{bb // 4}")
                    gt = gts.pop(bb)

                    # stage B: Y^T_b (e on partitions), d = K, W shared
                    yp = psum.tile([128, DCH * PSTRIDE], F32, tag="yp",
                                   name=f"yp{bb}")
                    for e in range(DCH):
                        for d in range(DCH):
                            nc.tensor.matmul(
                                yp[:, e * PSTRIDE:e * PSTRIDE + NPOS],
                                lhsT=w_sb[:, d * DIM + e * 128:
                                          d * DIM + e * 128 + 128],
                                rhs=gt[:, d * NPOS:(d + 1) * NPOS],
                                start=(d == 0),
                                stop=(d == DCH - 1),
                            )
                    # single strided eviction + bias, fp32->fp16, vector
                    nc.vector.tensor_add(
                        y_t[:, bi * YW:(bi + 1) * YW].rearrange(
                            "p (c u) -> p c u", c=DCH),
                        yp[:, :].rearrange("p (c s) -> p c s",
                                           c=DCH)[:, :, 0:NPOS],
                        bias,
                    )
                    for s0, ssz in YSTORES:
                        if bb == s0 + ssz - 1:
                            nc.sync.dma_start(
                                y_d[:, s0 * YW:(s0 + ssz) * YW],
                                y_t[:, (s0 % 4) * YW:
                                    (s0 % 4 + ssz) * YW])

    nc.compile()
    return nc


_PROGRAM = None


def _get_program():
    global _PROGRAM
    if _PROGRAM is None:
        _PROGRAM = build_program()
    return _PROGRAM


def _host_prep(x, Wv, qk, Wo, bo):
    x = np.asarray(x, dtype=np.float32)
    # per-core partition-major: [core, token, batch, feature] bf16
    XC = np.ascontiguousarray(
        x.reshape(N_CORES, BPC, NPOS, DIM).transpose(0, 2, 1, 3)
    ).astype(BF16_NP)
    W = (np.asarray(Wv, np.float32) @ np.asarray(Wo, np.float32)).astype(BF16_NP)
    # MT[(u,v),(p,q)] = qk[13+u-p, 13+v-q]: conv as a 196x196 matmul
    qk2 = np.asarray(qk, np.float32).reshape(KS, KS)
    idx = (KS // 2) + np.arange(IMG)[:, None] - np.arange(IMG)[None, :]
    MT = np.ascontiguousarray(
        qk2[idx[:, None, :, None], idx[None, :, None, :]].reshape(NPOS, NPOS)
    ).astype(BF16_NP)
    bo = np.asarray(bo, np.float32)
    mtb = np.zeros((128, 2 * NPOS + YW), dtype=BF16_NP)
    mtb[:, 0:NPOS] = MT[0:128, :]
    mtb[0:68, NPOS:2 * NPOS] = MT[128:NPOS, :]
    be = bo.reshape(DCH, 128).astype(BF16_NP)    # bias[c][p] = bo[128c+p]
    for c in range(DCH):
        mtb[:, 2 * NPOS + c * NPOS:2 * NPOS + (c + 1) * NPOS] = be[c][:, None]
    return XC, W, mtb


def _unpack_core(y2):
    # y2: [128, BPC*588] fp16 -> (BPC, NPOS, DIM) f32
    a = np.asarray(y2, np.float32).reshape(128, BPC, DCH, NPOS)
    # out[b, u, e=128c+p] = a[p, b, c, u]
    return np.ascontiguousarray(
        a.transpose(1, 3, 2, 0).reshape(BPC, NPOS, DIM))


def _run(x, Wv, qk, Wo, bo, **spmd_kwargs):
    XC, W, mtb = _host_prep(x, Wv, qk, Wo, bo)
    nc = _get_program()
    in_maps = [
        {"x": XC[c], "w": W, "mtb": mtb}
        for c in range(N_CORES)
    ]
    res = run_bass_kernel_spmd(nc, in_maps, list(range(N_CORES)), **spmd_kwargs)
    y = np.concatenate(
        [_unpack_core(res.results[c]["y"]) for c in range(N_CORES)], axis=0)
    return y, res


def kernel(x, Wv, qk, Wo, bo):
    y, _ = _run(x, Wv, qk, Wo, bo)
    return y
